# revision 1
# baseline (speedup 1.0000x reference)
"""Multi-head self-attention (B=2, N=2048, D=1024, H=16, Dh=64) on 8 TRN2 NeuronCores.

Sharding: core c handles batch b = c // 4 and head group g = c % 4 (heads 4g..4g+3).
Tensor-parallel on heads for qkv/out_proj; data-parallel on batch. Each core
produces a partial [D, N] output (transposed, bf16); host sums the 4 head-group
partials per batch in fp32, transposes, and adds b_out.

Single software-pipelined schedule: the exp of the softmax (ScalarE/ACT) is the
hard bottleneck (~16.8M elements/core at 128 lanes @ 1.2 GHz ~= 110us), so all
projection (qkv), output-projection, and DMA work is interleaved into the PE/DVE
slack underneath a continuous stream of 128 [128,1024] activations. On-chip data
is bf16 (halves DMA + SBUF); PSUM accumulation stays fp32.

PSUM budget (8 banks): scores 2x[128,1024] (4) + PV accum pA/pB (2) +
projection chain (1) + v-projection chain (1).
"""
import sys
import numpy as np

for _p in ("/opt/trn_rl_repo", "/root/.axon_site/_ro/trn_rl_repo"):
    if _p not in sys.path:
        sys.path.append(_p)

import ml_dtypes
import concourse.bass as bass
import concourse.bacc as bacc
import concourse.tile as tile
from concourse import mybir
from concourse.bass_utils import run_bass_kernel_spmd

F32 = mybir.dt.float32
BF16 = mybir.dt.bfloat16
EXP = mybir.ActivationFunctionType.Exp
ADD = mybir.AluOpType.add
MULT = mybir.AluOpType.mult

B, S, D = 2, 2048, 1024
H, DH = 16, 64
HL = 4            # heads per core
CQK = 512         # q+k channels per core
CV = 256          # v channels per core
ND = D // 128     # 8 d-tiles
NKT = S // 128    # 16 key tiles
NQC = S // 512    # 4 query chunks of 512
NW = 2 * NQC      # 8 attention windows: (qb, pair)
GS = NW * NKT     # 128 global pipeline steps


def build_kernel() -> "bass.Bass":
    nc = bacc.Bacc(None, target_bir_lowering=False, debug=False)

    xT = nc.dram_tensor("xT", [D, S], BF16, kind="ExternalInput")
    wqk = nc.dram_tensor("wqk", [D, CQK], BF16, kind="ExternalInput")
    bqk = nc.dram_tensor("bqk", [128, CQK // 128], F32, kind="ExternalInput")
    wv = nc.dram_tensor("wv", [D, CV], BF16, kind="ExternalInput")
    bvb = nc.dram_tensor("bvb", [128, CV], BF16, kind="ExternalInput")
    wout = nc.dram_tensor("wout", [CV, D], BF16, kind="ExternalInput")
    outT = nc.dram_tensor("outT", [D, S], BF16, kind="ExternalOutput")

    # partition-major views: one DMA moves a whole [128, t, cols] group
    xT_r = xT.rearrange("(t p) s -> t p s", p=128)        # [8, 128, 2048]
    wqk_p = wqk.rearrange("(t p) c -> p t c", p=128)      # [128, 8, 512]
    wv_p = wv.rearrange("(t p) c -> p t c", p=128)        # [128, 8, 256]
    wout_p = wout.rearrange("(t p) n -> p t n", p=128)    # [128, 2, 1024]
    outT_p = outT.rearrange("(t p) s -> p t s", p=128)    # [128, 8, 2048]

    with tile.TileContext(nc) as tc:
        with tc.tile_pool(name="persist", bufs=1) as persist, \
             tc.tile_pool(name="ptp", bufs=24) as ptp, \
             tc.tile_pool(name="stage", bufs=2) as stage, \
             tc.tile_pool(name="small", bufs=2) as small, \
             tc.tile_pool(name="ppj", bufs=1, space="PSUM") as ppj, \
             tc.tile_pool(name="pvp", bufs=1, space="PSUM") as pvp, \
             tc.tile_pool(name="psb", bufs=2, space="PSUM") as psb, \
             tc.tile_pool(name="pav", bufs=1, space="PSUM") as pav:

            # persistent SBUF tensors
            qkt_s = persist.tile([128, 4, S], BF16)       # ct: 0=q01 1=q23 2=k01 3=k23
            v_s = persist.tile([128, NKT, HL, DH + 2], BF16)  # per key-tile V + ones col
            at_s = persist.tile([128, 2, S], BF16)        # normalized attn out^T per pair
            xt_s = persist.tile([128, ND, S], BF16)
            wqk_s = persist.tile([128, ND, CQK], BF16)
            wv_s = persist.tile([128, ND, CV], BF16)
            wout_s = persist.tile([128, 2, D], BF16)
            bqk_s = persist.tile([128, CQK // 128], F32)
            bvb_s = persist.tile([128, CV], BF16)

            warm = persist.tile([128, 512], BF16)
            nc.vector.memset(warm[:], 0.25)
            nc.vector.memset(v_s[:, :, :, DH:DH + 1], 1.0)

            # ---- DMA emission ----
            # x moves as full d-major rows (4KB contiguous lines -> full HBM
            # bandwidth); the k01/q01 chains pipeline d-wise behind them.
            # Weights not needed until late (k23/q23/wout under p-outer window
            # order) are deferred so they don't steal bandwidth from x.
            # wqk col layout: 0:128=q01(m0) 128:256=q23(m1) 256:384=k01(m2) 384:512=k23(m3)
            nc.sync.dma_start(out=bqk_s[:], in_=bqk[:])
            nc.sync.dma_start(out=bvb_s[:], in_=bvb[:])
            nc.sync.dma_start(out=wqk_s[:, :, 256:384], in_=wqk_p[:, :, 256:384])
            nc.sync.dma_start(out=wqk_s[:, :, 0:128], in_=wqk_p[:, :, 0:128])
            for dd in range(ND):     # first x column-half: covers n0/n1 chains
                nc.sync.dma_start(out=xt_s[:, dd, 0:1024], in_=xT_r[dd][:, 0:1024])
            nc.sync.dma_start(out=wv_s[:], in_=wv_p[:])
            for dd in range(ND):
                nc.sync.dma_start(out=xt_s[:, dd, 1024:2048],
                                  in_=xT_r[dd][:, 1024:2048])
            nc.sync.dma_start(out=wqk_s[:, :, 128:256], in_=wqk_p[:, :, 128:256])
            nc.sync.dma_start(out=wqk_s[:, :, 384:512], in_=wqk_p[:, :, 384:512])
            nc.sync.dma_start(out=wout_s[:], in_=wout_p[:])

            # ---- deferred-work machinery ----
            # Emission order defines data semantics (readers see the last write
            # emitted before them), while bass_priority decides which READY
            # instruction an engine runs first. Background work is emitted
            # early (for correct dataflow edges) but at a large positive
            # priority offset, so it only fills engine slack and never
            # preempts the mainline score->exp->PV stream.
            LOWPRI = -(1 << 20)

            def emit_lowpri(ops):
                with tc.high_priority(offset=LOWPRI):
                    for _, fn in ops:
                        fn()

            def qk_chain_ops(m, n, ct, marker):
                """q/k projection chain: qkt_s[:, ct, n-block] = wqk_m^T @ x + b."""
                ops = []
                box = {}
                for d0 in range(0, ND, 2):
                    def f(d0=d0):
                        if d0 == 0:
                            box['ps'] = ppj.tile([128, 512], F32, tag="pj",
                                                 name=f"pj{m}{n}")
                        for d in (d0, d0 + 1):
                            nc.tensor.matmul(box['ps'][:],
                                             wqk_s[:, d, m * 128:(m + 1) * 128],
                                             xt_s[:, d, n * 512:(n + 1) * 512],
                                             start=(d == 0), stop=(d == ND - 1))
                    ops.append((None, f))

                def evac():
                    nc.vector.tensor_scalar_add(
                        qkt_s[:, ct, n * 512:(n + 1) * 512],
                        box['ps'][:], bqk_s[:, m:m + 1])
                ops.append((marker, evac))
                return ops

            def _v_evac(ps, st):
                nc.vector.tensor_tensor(
                    out=v_s[:, st, :, 0:DH],
                    in0=ps[:, 0:256].rearrange("p (h c) -> p h c", h=HL),
                    in1=bvb_s[:].rearrange("p (h c) -> p h c", h=HL),
                    op=ADD)

            def v_pair_ops(st0):
                """v projection for key tiles st0, st0+1 (sequential full-bank
                chains in the v bank)."""
                ops = []
                for j in (0, 1):
                    st = st0 + j
                    box = {}
                    for d0 in range(0, ND, 2):
                        def f(d0=d0, st=st, box=box):
                            if d0 == 0:
                                box['ps'] = pvp.tile([128, 512], F32, tag="v",
                                                     name=f"pv{st}")
                            for d in (d0, d0 + 1):
                                nc.tensor.matmul(box['ps'][:, 0:256],
                                                 xt_s[:, d, st * 128:(st + 1) * 128],
                                                 wv_s[:, d, :],
                                                 start=(d == 0), stop=(d == ND - 1))
                        ops.append((None, f))
                    ops.append((f"v{st}", lambda st=st, box=box: _v_evac(box['ps'], st)))
                return ops

            def v_pair_dual_ops(st0):
                """v projection for key tiles st0, st0+1 concurrently, using the
                pj and v banks (only when no projection chain is pending)."""
                ops = []
                box = {}
                for d in range(ND):
                    def f(d=d):
                        if d == 0:
                            box[0] = ppj.tile([128, 512], F32, tag="pj",
                                              name=f"pv{st0}")
                            box[1] = pvp.tile([128, 512], F32, tag="v",
                                              name=f"pv{st0 + 1}")
                        for j in (0, 1):
                            st = st0 + j
                            nc.tensor.matmul(box[j][:, 0:256],
                                             xt_s[:, d, st * 128:(st + 1) * 128],
                                             wv_s[:, d, :],
                                             start=(d == 0), stop=(d == ND - 1))
                    ops.append((None, f))
                for j in (0, 1):
                    ops.append((f"v{st0 + j}",
                                lambda j=j: _v_evac(box[j], st0 + j)))
                return ops

            ostage = {}

            def c_ops(qc):
                """out^T[:, qc-block] = wout^T @ at: 8 nt chains (ct-chained)
                into a staged [128, 8, 512] tile, then one batched DMA out."""
                ops = []
                obox = {}
                for nt in range(ND):
                    box = {}

                    def mm(nt=nt, box=box):
                        pool, tg = (ppj, "pj") if nt % 2 == 0 else (pvp, "v")
                        box['ps'] = pool.tile([128, 512], F32, tag=tg,
                                              name=f"pc{qc}{nt}")
                        for ct in (0, 1):
                            nc.tensor.matmul(box['ps'][:],
                                             wout_s[:, ct, nt * 128:(nt + 1) * 128],
                                             at_s[:, ct, qc * 512:(qc + 1) * 512],
                                             start=(ct == 0), stop=(ct == 1))

                    def ev(nt=nt, box=box):
                        if nt == 0:
                            obox['o'] = stage.tile([128, ND, 512], BF16, tag="o",
                                                   name=f"o{qc}")
                        nc.vector.tensor_copy(out=obox['o'][:, nt, :],
                                              in_=box['ps'][:])
                    ops.append((None, mm))
                    ops.append((None, ev))

                def out_dma():
                    nc.sync.dma_start(
                        out=outT_p[:, :, qc * 512:(qc + 1) * 512], in_=obox['o'][:])
                ops.append((None, out_dma))
                return ops

            def c3_first_ops(qc):
                """ct=0 half of a late C chunk: runs as soon as the p=0 at-rows
                for qb=qc are complete (window qc under p-outer order)."""
                ops = []
                o3 = stage.tile([128, ND, 512], F32, tag="o3", name=f"o3_{qc}",
                                bufs=1)
                ostage[qc] = o3
                for nt in range(ND):
                    box = {}

                    def mm(nt=nt, box=box):
                        pool, tg = (ppj, "pj") if nt % 2 == 0 else (pvp, "v")
                        box['ps'] = pool.tile([128, 512], F32, tag=tg,
                                              name=f"pc3a{nt}")
                        nc.tensor.matmul(box['ps'][:],
                                         wout_s[:, 0, nt * 128:(nt + 1) * 128],
                                         at_s[:, 0, qc * 512:(qc + 1) * 512],
                                         start=True, stop=True)

                    def ev(nt=nt, box=box):
                        nc.vector.tensor_copy(out=o3[:, nt, :], in_=box['ps'][:])
                    ops.append((None, mm))
                    ops.append((None, ev))
                return ops

            def c3_second_ops(qc, tail):
                """ct=1 half of a late C chunk + combine + out. In the tail the
                dead score banks join the psum rotation; mid-stream they stay
                reserved for the exp pipeline."""
                ops = []
                obox = {}
                if tail:
                    # tail warm-ups: keep the PE HAM-warm through the final
                    # normalize so the last C matmuls run at full clock
                    for i in range(14):
                        def wf(i=i):
                            tg = "pA" if i % 2 == 0 else "pB"
                            wps = pav.tile([128, 512], F32, tag=tg,
                                           name=f"twarm{i}")
                            nc.tensor.matmul(wps[:], warm[:, 0:128], warm[:],
                                             start=True, stop=True)
                        ops.append((None, wf))
                    slots = [(ppj, "pj"), (pvp, "v"), (psb, "sAB"), (psb, "sAB")]
                else:
                    slots = [(ppj, "pj"), (pvp, "v")]
                for nt in range(ND):
                    box = {}

                    def mm(nt=nt, box=box):
                        pool, tg = slots[nt % len(slots)]
                        box['ps'] = pool.tile([128, 512], F32, tag=tg,
                                              name=f"pc3b{qc}{nt}")
                        nc.tensor.matmul(box['ps'][:],
                                         wout_s[:, 1, nt * 128:(nt + 1) * 128],
                                         at_s[:, 1, qc * 512:(qc + 1) * 512],
                                         start=True, stop=True)

                    def ev(nt=nt, box=box):
                        if nt == 0:
                            obox['o'] = stage.tile([128, ND, 512], BF16, tag="o",
                                                   name=f"o{qc}")
                        nc.vector.tensor_tensor(out=obox['o'][:, nt, :],
                                                in0=ostage[qc][:, nt, :],
                                                in1=box['ps'][:], op=ADD)
                        if nt == 3 or nt == ND - 1:
                            lo = 0 if nt == 3 else 4
                            nc.sync.dma_start(
                                out=outT_p[:, lo:nt + 1, qc * 512:(qc + 1) * 512],
                                in_=obox['o'][:, lo:nt + 1, :])
                    ops.append((None, mm))
                    ops.append((None, ev))
                return ops

            # HAM warm-up at low priority: fills the PE-idle DMA wait at the
            # start so the first real chains run at 2.4 GHz, and yields the
            # moment real work is ready.
            with tc.high_priority(offset=LOWPRI):
                for i in range(24):
                    wps = psb.tile([128, 512], F32, tag="sAB", name=f"warm{i}")
                    nc.tensor.matmul(wps[:], warm[:, 0:128], warm[:],
                                     start=True, stop=True)

            # ---- head: minimum work before the first exp can fire ----
            # k01 n0 and q01 n0 interleaved d-wise so both pipeline behind the
            # x d-tile DMA stream.
            hbox = {}
            for d in range(ND):
                if d == 0:
                    hbox['k'] = ppj.tile([128, 512], F32, tag="pj", name="hk01")
                    hbox['q'] = pvp.tile([128, 512], F32, tag="v", name="hq01")
                nc.tensor.matmul(hbox['k'][:], wqk_s[:, d, 256:384],
                                 xt_s[:, d, 0:512],
                                 start=(d == 0), stop=(d == ND - 1))
                nc.tensor.matmul(hbox['q'][:], wqk_s[:, d, 0:128],
                                 xt_s[:, d, 0:512],
                                 start=(d == 0), stop=(d == ND - 1))
            nc.vector.tensor_scalar_add(qkt_s[:, 2, 0:512], hbox['k'][:],
                                        bqk_s[:, 2:3])
            nc.vector.tensor_scalar_add(qkt_s[:, 0, 0:512], hbox['q'][:],
                                        bqk_s[:, 0:1])
            for st0 in (0, 2):               # v tiles 0..3
                for _, f in v_pair_ops(st0):
                    f()

            # ---- background work: emitted now (defines dataflow), low priority ----
            # Window order is p-outer ((qb,p) = (w%4, w//4)), so k23/q23 are
            # not needed until window 4 — the projection load spreads evenly.
            # S-gating chains first (exps cannot cross a window boundary
            # without them); v-projection work trails behind, buffered by the
            # deep pt pool (PVs may lag the exp stream by up to 24 steps).
            emit_lowpri(qk_chain_ops(2, 1, 2, "k01n1"))
            emit_lowpri(qk_chain_ops(2, 2, 2, "k01n2"))
            emit_lowpri(qk_chain_ops(2, 3, 2, "k01n3"))
            emit_lowpri(qk_chain_ops(0, 1, 0, "q01n1"))
            emit_lowpri(v_pair_ops(4))
            emit_lowpri(qk_chain_ops(0, 2, 0, "q01n2"))
            emit_lowpri(v_pair_ops(6))
            emit_lowpri(v_pair_dual_ops(8))
            emit_lowpri(qk_chain_ops(0, 3, 0, "q01n3"))
            emit_lowpri(v_pair_dual_ops(10))
            emit_lowpri(v_pair_dual_ops(12))
            emit_lowpri(v_pair_dual_ops(14))
            emit_lowpri(qk_chain_ops(3, 0, 3, "k23n0"))
            emit_lowpri(qk_chain_ops(1, 0, 1, "q23n0"))
            emit_lowpri(qk_chain_ops(3, 1, 3, "k23n1"))
            emit_lowpri(qk_chain_ops(1, 1, 1, "q23n1"))
            emit_lowpri(qk_chain_ops(3, 2, 3, "k23n2"))
            emit_lowpri(qk_chain_ops(3, 3, 3, "k23n3"))
            emit_lowpri(qk_chain_ops(1, 2, 1, "q23n2"))
            emit_lowpri(qk_chain_ops(1, 3, 1, "q23n3"))
            # C(qc) ops are emitted once at_s for qb=qc is complete.

            sabs = {}
            pAB = [None, None]

            def emit_S(g):
                w, t = divmod(g, NKT)
                p, qb = divmod(w, NQC)
                qs = slice(qb * 512, qb * 512 + 512)
                qt = qkt_s[:, p, :]
                kt = qkt_s[:, 2 + p, :]
                sAB = psb.tile([128, 1024], F32, tag="sAB", name=f"sAB{g}")
                nc.tensor.matmul(sAB[:, 0:512],
                                 kt[0:64, t * 128:(t + 1) * 128],
                                 qt[0:64, qs], start=True, stop=True,
                                 tile_position=(0, 0))
                nc.tensor.matmul(sAB[:, 512:1024],
                                 kt[64:128, t * 128:(t + 1) * 128],
                                 qt[64:128, qs], start=True, stop=True,
                                 tile_position=(64, 0))
                sabs[g] = sAB

            def normalize(w):
                p, qb = divmod(w, NQC)
                qs = slice(qb * 512, qb * 512 + 512)
                # last window: run the DMA-bounce half first to shorten the tail
                locs = ((1, pAB[1]), (0, pAB[0])) if w == NW - 1 else \
                       ((0, pAB[0]), (1, pAB[1]))
                for loc, pX in locs:
                    raw = small.tile([DH + 1, 512], F32, tag="raw",
                                     name=f"raw{w}{loc}")
                    nc.vector.tensor_copy(out=raw[:], in_=pX[:])
                    dn = small.tile([64, 8], F32, tag="dn", name="dn")
                    nc.sync.dma_start(out=dn[:], in_=raw[DH:DH + 1, :])
                    rr = small.tile([64, 8], F32, tag="rr", name="rr")
                    nc.vector.reciprocal(rr[:], dn[:])
                    r = small.tile([1, 512], F32, tag="r", name="r")
                    nc.sync.dma_start(out=r[:], in_=rr[:])
                    rb = small.tile([64, 512], F32, tag="rb", name="rb")
                    nc.gpsimd.partition_broadcast(rb[:], r[:])
                    if loc == 0:
                        nc.vector.tensor_tensor(out=at_s[0:64, p, qs],
                                                in0=raw[0:DH, :], in1=rb[:],
                                                op=MULT)
                    else:
                        # DVE lanes cannot shift partitions; bounce via DMA
                        tmp = small.tile([64, 512], BF16, tag="tmp", name="tmp")
                        nc.vector.tensor_tensor(out=tmp[:], in0=raw[0:DH, :],
                                                in1=rb[:], op=MULT)
                        nc.sync.dma_start(out=at_s[64:128, p, qs], in_=tmp[:])

            # ---- the fused pipeline ----
            emit_S(0)
            for g in range(GS):
                w, t = divmod(g, NKT)
                p, qb = divmod(w, NQC)
                if g + 1 < GS:
                    emit_S(g + 1)
                pt = ptp.tile([128, 1024], BF16, tag="pt", name=f"pt{g}")
                nc.scalar.activation(pt[:], sabs[g][:], EXP)
                del sabs[g]
                if t == 0:
                    pAB[0] = pav.tile([DH + 1, 512], F32, tag="pA", name=f"pA{w}")
                    pAB[1] = pav.tile([DH + 1, 512], F32, tag="pB", name=f"pB{w}")
                nc.tensor.matmul(pAB[0][:], v_s[:, t, 2 * p, 0:DH + 1],
                                 pt[:, 0:512],
                                 start=(t == 0), stop=(t == NKT - 1))
                nc.tensor.matmul(pAB[1][:], v_s[:, t, 2 * p + 1, 0:DH + 1],
                                 pt[:, 512:1024],
                                 start=(t == 0), stop=(t == NKT - 1))
                if t == NKT - 1:
                    normalize(w)
                    if p == 0 and qb == NQC - 1:
                        # at[:, 0, q3] complete -> ct=0 half of the last C chunk
                        emit_lowpri(c3_first_ops(NQC - 1))
                    elif p == 1 and qb < NQC - 1:
                        emit_lowpri(c_ops(qb))
                    elif p == 1 and qb == NQC - 1:
                        emit_lowpri(c3_second_ops(NQC - 1, tail=True))
    nc.compile()
    return nc


def shard_inputs(x, W_qkv, b_qkv, W_out, b_out=None):
    """Build the 8 per-core input maps. Core c: batch c//4, head group c%4."""
    in_maps = []
    scale = 1.0 / np.sqrt(np.float32(DH))
    bf16 = ml_dtypes.bfloat16
    for c in range(8):
        b, g = divmod(c, 4)
        cs = slice(g * 256, g * 256 + 256)
        xTc = np.ascontiguousarray(x[b].T)                       # [D, S]
        wq = W_qkv[:, 0:D][:, cs] * scale                        # [D, 256]
        wk = W_qkv[:, D:2 * D][:, cs]
        wqkc = np.ascontiguousarray(np.concatenate([wq, wk], axis=1))  # [D, 512]
        bq = b_qkv[0:D][cs] * scale
        bk = b_qkv[D:2 * D][cs]
        bqkc = np.concatenate([bq, bk]).reshape(CQK // 128, 128).T     # [128, 4]
        wvc = np.ascontiguousarray(W_qkv[:, 2 * D:3 * D][:, cs])       # [D, 256]
        bvbc = np.ascontiguousarray(
            np.broadcast_to(b_qkv[2 * D:3 * D][cs], (128, CV)))        # [128, 256]
        woutc = np.ascontiguousarray(W_out[cs, :])                     # [256, D]
        in_maps.append({
            "xT": xTc.astype(bf16),
            "wqk": wqkc.astype(bf16),
            "bqk": np.ascontiguousarray(bqkc).astype(np.float32),
            "wv": wvc.astype(bf16),
            "bvb": bvbc.astype(bf16),
            "wout": woutc.astype(bf16),
        })
    return in_maps


_NC_CACHE = []


def _get_nc():
    if not _NC_CACHE:
        _NC_CACHE.append(build_kernel())
    return _NC_CACHE[0]


def run_sharded(in_maps, **kwargs):
    nc = _get_nc()
    return run_bass_kernel_spmd(nc, in_maps, core_ids=list(range(8)), **kwargs)


def gather_output(results, b_out):
    out = np.empty((B, S, D), dtype=np.float32)
    for b in range(B):
        acc = np.asarray(results[4 * b]["outT"], dtype=np.float32).copy()
        for g in range(1, 4):
            acc += np.asarray(results[4 * b + g]["outT"], dtype=np.float32)
        out[b] = acc.T + b_out[None, :]
    return out


def kernel(x, W_qkv, b_qkv, W_out, b_out):
    x = np.asarray(x, dtype=np.float32)
    W_qkv = np.asarray(W_qkv, dtype=np.float32)
    b_qkv = np.asarray(b_qkv, dtype=np.float32)
    W_out = np.asarray(W_out, dtype=np.float32)
    b_out = np.asarray(b_out, dtype=np.float32)
    in_maps = shard_inputs(x=x, W_qkv=W_qkv, b_qkv=b_qkv, W_out=W_out, b_out=b_out)
    res = run_sharded(in_maps)
    return gather_output(res.results, b_out)



# revision 3
# speedup vs baseline: 1.0424x; 1.0424x over previous
"""Multi-head self-attention (B=2, N=2048, D=1024, H=16, Dh=64) on 8 TRN2 NeuronCores.

Sharding: core c handles batch b = c // 4 and head group g = c % 4 (heads 4g..4g+3).
Tensor-parallel on heads for qkv/out_proj; data-parallel on batch. Each core
produces a partial [D, N] output (transposed, bf16); host sums the 4 head-group
partials per batch in fp32, transposes, and adds b_out.

Single software-pipelined schedule: the exp of the softmax (ScalarE/ACT) is the
hard bottleneck (~16.8M elements/core -> ~142us of ACT busy at FD=1024), so all
projection (qkv), output-projection, and DMA work is interleaved into the PE/DVE
slack underneath a continuous stream of 128 [128,1024] activations. On-chip data
is bf16 (halves DMA + SBUF); PSUM accumulation stays fp32.

v2 scheduling changes vs v1 (213us):
 - S (score) matmuls at high priority: the static Tile scheduler otherwise
   drains the PV backlog (emitted earlier, same priority) ahead of the next
   S pair and starves the ACT exp stream mid-window.
 - Window order (p,qb): p0 q0..q3 then p1 q3,q0,q1,q2 -- at[:, :, q3] is the
   first complete column block of the second half, so ALL four out-projection
   chunks run mid-stream as plain c_ops; the old 31us tail (final normalize +
   ct=1 C half + HAM re-throttle) collapses to normalize + one C chunk.
 - Head: x is DMA'd in 512-col slices so the k01n0/q01n0 chains pipeline
   d-wise right behind the first 8 slice DMAs; only v tiles 0-1 are built
   inline (the 24-deep pt pool makes later v tiles soft-deadline); first exp
   fires ~12us in (preamble ~6us is fixed runtime cost) vs 25us in v1.

PSUM budget (8 banks): scores 2x[128,1024] (4) + PV accum pA/pB (2) +
projection chain (1) + v-projection chain (1).
"""
import sys
import numpy as np

for _p in ("/opt/trn_rl_repo", "/root/.axon_site/_ro/trn_rl_repo"):
    if _p not in sys.path:
        sys.path.append(_p)

import ml_dtypes
import concourse.bass as bass
import concourse.bacc as bacc
import concourse.tile as tile
from concourse import mybir
from concourse.bass_utils import run_bass_kernel_spmd

F32 = mybir.dt.float32
BF16 = mybir.dt.bfloat16
EXP = mybir.ActivationFunctionType.Exp
ADD = mybir.AluOpType.add
MULT = mybir.AluOpType.mult

B, S, D = 2, 2048, 1024
H, DH = 16, 64
HL = 4            # heads per core
CQK = 512         # q+k channels per core
CV = 256          # v channels per core
ND = D // 128     # 8 d-tiles
NKT = S // 128    # 16 key tiles
NQC = S // 512    # 4 query chunks of 512
NW = 2 * NQC      # 8 attention windows: (p, qb)
GS = NW * NKT     # 128 global pipeline steps

# window order: p=0 ascending qb, then p=1 with qb=3 FIRST (so the last C
# chunk runs mid-stream, not in the tail)
WORDER = [(0, 0), (0, 1), (0, 2), (0, 3), (1, 3), (1, 0), (1, 1), (1, 2)]


def build_kernel() -> "bass.Bass":
    nc = bacc.Bacc(None, target_bir_lowering=False, debug=False)

    xT = nc.dram_tensor("xT", [D, S], BF16, kind="ExternalInput")
    wqk = nc.dram_tensor("wqk", [D, CQK], BF16, kind="ExternalInput")
    bqk = nc.dram_tensor("bqk", [128, CQK // 128], F32, kind="ExternalInput")
    wv = nc.dram_tensor("wv", [D, CV], BF16, kind="ExternalInput")
    bvb = nc.dram_tensor("bvb", [128, CV], BF16, kind="ExternalInput")
    wout = nc.dram_tensor("wout", [CV, D], BF16, kind="ExternalInput")
    outT = nc.dram_tensor("outT", [D, S], BF16, kind="ExternalOutput")

    # partition-major views: one DMA moves a whole [128, t, cols] group
    xT_r = xT.rearrange("(t p) s -> t p s", p=128)        # [8, 128, 2048]
    wqk_p = wqk.rearrange("(t p) c -> p t c", p=128)      # [128, 8, 512]
    wv_p = wv.rearrange("(t p) c -> p t c", p=128)        # [128, 8, 256]
    wout_p = wout.rearrange("(t p) n -> p t n", p=128)    # [128, 2, 1024]
    outT_p = outT.rearrange("(t p) s -> p t s", p=128)    # [128, 8, 2048]

    with tile.TileContext(nc) as tc:
        with tc.tile_pool(name="persist", bufs=1) as persist, \
             tc.tile_pool(name="ptp", bufs=24) as ptp, \
             tc.tile_pool(name="stage", bufs=2) as stage, \
             tc.tile_pool(name="small", bufs=2) as small, \
             tc.tile_pool(name="ppj", bufs=1, space="PSUM") as ppj, \
             tc.tile_pool(name="pvp", bufs=1, space="PSUM") as pvp, \
             tc.tile_pool(name="psb", bufs=2, space="PSUM") as psb, \
             tc.tile_pool(name="pav", bufs=1, space="PSUM") as pav:

            # persistent SBUF tensors
            qkt_s = persist.tile([128, 4, S], BF16)       # ct: 0=q01 1=q23 2=k01 3=k23
            v_s = persist.tile([128, NKT, HL, DH + 2], BF16)  # per key-tile V + ones col
            at_s = persist.tile([128, 2, S], BF16)        # normalized attn out^T per p
            xt_s = persist.tile([128, ND, S], BF16)
            wqk_s = persist.tile([128, ND, CQK], BF16)
            wv_s = persist.tile([128, ND, CV], BF16)
            wout_s = persist.tile([128, 2, D], BF16)
            bqk_s = persist.tile([128, CQK // 128], F32)
            bvb_s = persist.tile([128, CV], BF16)

            warm = persist.tile([128, 512], BF16)
            nc.vector.memset(warm[:], 0.25)
            nc.vector.memset(v_s[:, :, :, DH:DH + 1], 1.0)

            # priority levels (positive offset => appears earlier to scheduler)
            HIPRI = 1 << 19       # mainline S matmuls (feed the exp stream)
            EVPRI = 1 << 18       # chain evacs + normalize (gate S windows / C)
            LOWPRI = -(1 << 20)   # background projection/C work
            LOW2 = -(1 << 21)     # HAM warm-up filler

            def lp(fn):
                with tc.high_priority(offset=LOWPRI):
                    fn()

            def ep(fn):
                with tc.high_priority(offset=EVPRI):
                    fn()

            # ---- DMA emission ----
            # Order = need order: q01/k01 weights + x n0-block slices first so
            # the head chains pipeline d-wise right behind them; v weights next
            # (v tiles stream through window 0); the rest paced behind.
            # wqk col layout: 0:128=q01(m0) 128:256=q23(m1) 256:384=k01(m2) 384:512=k23(m3)
            nc.sync.dma_start(out=bqk_s[:], in_=bqk[:])
            nc.sync.dma_start(out=wqk_s[:, :, 0:128], in_=wqk_p[:, :, 0:128])
            nc.sync.dma_start(out=wqk_s[:, :, 256:384], in_=wqk_p[:, :, 256:384])
            for dd in range(ND):     # x n0 block: feeds k01n0/q01n0 + v0..v3
                nc.sync.dma_start(out=xt_s[:, dd, 0:512], in_=xT_r[dd][:, 0:512])
            nc.sync.dma_start(out=bvb_s[:], in_=bvb[:])
            nc.sync.dma_start(out=wv_s[:], in_=wv_p[:])
            for dd in range(ND):
                nc.sync.dma_start(out=xt_s[:, dd, 512:1024],
                                  in_=xT_r[dd][:, 512:1024])
            nc.sync.dma_start(out=wqk_s[:, :, 384:512], in_=wqk_p[:, :, 384:512])
            for dd in range(ND):
                nc.sync.dma_start(out=xt_s[:, dd, 1024:1536],
                                  in_=xT_r[dd][:, 1024:1536])
            nc.sync.dma_start(out=wqk_s[:, :, 128:256], in_=wqk_p[:, :, 128:256])
            for dd in range(ND):
                nc.sync.dma_start(out=xt_s[:, dd, 1536:2048],
                                  in_=xT_r[dd][:, 1536:2048])
            nc.sync.dma_start(out=wout_s[:], in_=wout_p[:])

            # ---- chain builders ----
            def qk_chain_ops(m, n, ct, marker):
                """q/k projection chain: qkt_s[:, ct, n-block] = wqk_m^T @ x + b.
                Matmuls run at LOWPRI; the evac (which gates S windows) at EVPRI."""
                box = {}
                for d0 in range(0, ND, 2):
                    def f(d0=d0):
                        if d0 == 0:
                            box['ps'] = ppj.tile([128, 512], F32, tag="pj",
                                                 name=f"pj{m}{n}")
                        for d in (d0, d0 + 1):
                            nc.tensor.matmul(box['ps'][:],
                                             wqk_s[:, d, m * 128:(m + 1) * 128],
                                             xt_s[:, d, n * 512:(n + 1) * 512],
                                             start=(d == 0), stop=(d == ND - 1))
                    lp(f)

                def evac():
                    nc.vector.tensor_scalar_add(
                        qkt_s[:, ct, n * 512:(n + 1) * 512],
                        box['ps'][:], bqk_s[:, m:m + 1])
                ep(evac)

            def _v_evac(ps, st):
                nc.vector.tensor_tensor(
                    out=v_s[:, st, :, 0:DH],
                    in0=ps[:, 0:256].rearrange("p (h c) -> p h c", h=HL),
                    in1=bvb_s[:].rearrange("p (h c) -> p h c", h=HL),
                    op=ADD)

            def v_pair(st0, inline=False):
                """v projection for key tiles st0, st0+1 (sequential full-bank
                chains in the v bank)."""
                run = (lambda f: f()) if inline else lp
                for j in (0, 1):
                    st = st0 + j
                    box = {}
                    for d0 in range(0, ND, 2):
                        def f(d0=d0, st=st, box=box):
                            if d0 == 0:
                                box['ps'] = pvp.tile([128, 512], F32, tag="v",
                                                     name=f"pv{st}")
                            for d in (d0, d0 + 1):
                                nc.tensor.matmul(box['ps'][:, 0:256],
                                                 xt_s[:, d, st * 128:(st + 1) * 128],
                                                 wv_s[:, d, :],
                                                 start=(d == 0), stop=(d == ND - 1))
                        run(f)
                    ep(lambda st=st, box=box: _v_evac(box['ps'], st))

            def v_pair_dual(st0):
                """v projection for key tiles st0, st0+1 concurrently, using the
                pj and v banks (only when no qk chain is imminent)."""
                box = {}
                for d in range(ND):
                    def f(d=d):
                        if d == 0:
                            box[0] = ppj.tile([128, 512], F32, tag="pj",
                                              name=f"pv{st0}")
                            box[1] = pvp.tile([128, 512], F32, tag="v",
                                              name=f"pv{st0 + 1}")
                        for j in (0, 1):
                            st = st0 + j
                            nc.tensor.matmul(box[j][:, 0:256],
                                             xt_s[:, d, st * 128:(st + 1) * 128],
                                             wv_s[:, d, :],
                                             start=(d == 0), stop=(d == ND - 1))
                    lp(f)
                for j in (0, 1):
                    ep(lambda j=j: _v_evac(box[j], st0 + j))

            def c_ops(qc):
                """out^T[:, qc-block] = wout^T @ at: 8 nt chains (ct-chained)
                into a staged [128, 8, 512] tile, DMA'd out in two halves."""
                obox = {}
                for nt in range(ND):
                    box = {}

                    def mm(nt=nt, box=box):
                        pool, tg = (ppj, "pj") if nt % 2 == 0 else (pvp, "v")
                        box['ps'] = pool.tile([128, 512], F32, tag=tg,
                                              name=f"pc{qc}{nt}")
                        for ct in (0, 1):
                            nc.tensor.matmul(box['ps'][:],
                                             wout_s[:, ct, nt * 128:(nt + 1) * 128],
                                             at_s[:, ct, qc * 512:(qc + 1) * 512],
                                             start=(ct == 0), stop=(ct == 1))

                    def ev(nt=nt, box=box):
                        if nt == 0:
                            obox['o'] = stage.tile([128, ND, 512], BF16, tag="o",
                                                   name=f"o{qc}")
                        nc.vector.tensor_copy(out=obox['o'][:, nt, :],
                                              in_=box['ps'][:])
                        if nt == 3 or nt == ND - 1:
                            lo = 0 if nt == 3 else 4
                            nc.sync.dma_start(
                                out=outT_p[:, lo:nt + 1, qc * 512:(qc + 1) * 512],
                                in_=obox['o'][:, lo:nt + 1, :])
                    lp(mm)
                    lp(ev)

            # HAM warm-up at lowest priority: fills the PE-idle DMA wait at the
            # start so the first real chains run at 2.4 GHz, and yields the
            # moment real work is ready.
            with tc.high_priority(offset=LOW2):
                for i in range(8):
                    wps = psb.tile([128, 512], F32, tag="sAB", name=f"warm{i}")
                    nc.tensor.matmul(wps[:], warm[:, 0:128], warm[:],
                                     start=True, stop=True)

            # ---- head: minimum work before the first exp can fire ----
            # k01 n0 and q01 n0 interleaved d-wise so both pipeline behind the
            # x slice DMA stream.
            hbox = {}
            for d in range(ND):
                if d == 0:
                    hbox['k'] = ppj.tile([128, 512], F32, tag="pj", name="hk01")
                    hbox['q'] = pvp.tile([128, 512], F32, tag="v", name="hq01")
                nc.tensor.matmul(hbox['k'][:], wqk_s[:, d, 256:384],
                                 xt_s[:, d, 0:512],
                                 start=(d == 0), stop=(d == ND - 1))
                nc.tensor.matmul(hbox['q'][:], wqk_s[:, d, 0:128],
                                 xt_s[:, d, 0:512],
                                 start=(d == 0), stop=(d == ND - 1))
            nc.vector.tensor_scalar_add(qkt_s[:, 2, 0:512], hbox['k'][:],
                                        bqk_s[:, 2:3])
            nc.vector.tensor_scalar_add(qkt_s[:, 0, 0:512], hbox['q'][:],
                                        bqk_s[:, 0:1])
            v_pair(0, inline=True)           # v tiles 0,1

            # ---- background work: emitted now (defines dataflow), low priority ----
            # Deadlines (stream step ~1.15us): k01nX by step 4X; v tiles are
            # soft (pt pool buffers ~24 steps); q01nX by window X; k23/q23n3 by
            # window 4 (step 64); q23n0..2 by windows 5..7.
            qk_chain_ops(2, 1, 2, "k01n1")
            v_pair(2)
            qk_chain_ops(2, 2, 2, "k01n2")
            v_pair(4)
            qk_chain_ops(2, 3, 2, "k01n3")
            v_pair(6)
            qk_chain_ops(0, 1, 0, "q01n1")
            v_pair_dual(8)
            v_pair_dual(10)
            qk_chain_ops(0, 2, 0, "q01n2")
            v_pair_dual(12)
            v_pair_dual(14)
            qk_chain_ops(0, 3, 0, "q01n3")
            qk_chain_ops(3, 0, 3, "k23n0")
            qk_chain_ops(1, 3, 1, "q23n3")
            qk_chain_ops(3, 1, 3, "k23n1")
            qk_chain_ops(3, 2, 3, "k23n2")
            qk_chain_ops(3, 3, 3, "k23n3")
            qk_chain_ops(1, 0, 1, "q23n0")
            qk_chain_ops(1, 1, 1, "q23n1")
            qk_chain_ops(1, 2, 1, "q23n2")
            # c_ops(qc) emitted once at_s for qb=qc is complete (second half).

            sabs = {}
            pAB = [None, None]

            def emit_S(g):
                w, t = divmod(g, NKT)
                p, qb = WORDER[w]
                qs = slice(qb * 512, qb * 512 + 512)
                qt = qkt_s[:, p, :]
                kt = qkt_s[:, 2 + p, :]
                with tc.high_priority(offset=HIPRI):
                    sAB = psb.tile([128, 1024], F32, tag="sAB", name=f"sAB{g}")
                    nc.tensor.matmul(sAB[:, 0:512],
                                     kt[0:64, t * 128:(t + 1) * 128],
                                     qt[0:64, qs], start=True, stop=True,
                                     tile_position=(0, 0))
                    nc.tensor.matmul(sAB[:, 512:1024],
                                     kt[64:128, t * 128:(t + 1) * 128],
                                     qt[64:128, qs], start=True, stop=True,
                                     tile_position=(64, 0))
                sabs[g] = sAB

            def normalize(w):
                p, qb = WORDER[w]
                qs = slice(qb * 512, qb * 512 + 512)
                # last window: run the DMA-bounce half first to shorten the tail
                locs = ((1, pAB[1]), (0, pAB[0])) if w == NW - 1 else \
                       ((0, pAB[0]), (1, pAB[1]))
                with tc.high_priority(offset=EVPRI):
                    for loc, pX in locs:
                        raw = small.tile([DH + 1, 512], F32, tag="raw",
                                         name=f"raw{w}{loc}")
                        nc.vector.tensor_copy(out=raw[:], in_=pX[:])
                        dn = small.tile([64, 8], F32, tag="dn", name="dn")
                        nc.sync.dma_start(out=dn[:], in_=raw[DH:DH + 1, :])
                        rr = small.tile([64, 8], F32, tag="rr", name="rr")
                        nc.vector.reciprocal(rr[:], dn[:])
                        r = small.tile([1, 512], F32, tag="r", name="r")
                        nc.sync.dma_start(out=r[:], in_=rr[:])
                        rb = small.tile([64, 512], F32, tag="rb", name="rb")
                        nc.gpsimd.partition_broadcast(rb[:], r[:])
                        if loc == 0:
                            nc.vector.tensor_tensor(out=at_s[0:64, p, qs],
                                                    in0=raw[0:DH, :], in1=rb[:],
                                                    op=MULT)
                        else:
                            # DVE lanes cannot shift partitions; bounce via DMA
                            tmp = small.tile([64, 512], BF16, tag="tmp",
                                             name="tmp")
                            nc.vector.tensor_tensor(out=tmp[:], in0=raw[0:DH, :],
                                                    in1=rb[:], op=MULT)
                            nc.sync.dma_start(out=at_s[64:128, p, qs], in_=tmp[:])

            # ---- the fused pipeline ----
            emit_S(0)
            for g in range(GS):
                w, t = divmod(g, NKT)
                p, qb = WORDER[w]
                if g + 1 < GS:
                    emit_S(g + 1)
                pt = ptp.tile([128, 1024], BF16, tag="pt", name=f"pt{g}")
                nc.scalar.activation(pt[:], sabs[g][:], EXP)
                del sabs[g]
                if t == 0:
                    pAB[0] = pav.tile([DH + 1, 512], F32, tag="pA", name=f"pA{w}")
                    pAB[1] = pav.tile([DH + 1, 512], F32, tag="pB", name=f"pB{w}")
                nc.tensor.matmul(pAB[0][:], v_s[:, t, 2 * p, 0:DH + 1],
                                 pt[:, 0:512],
                                 start=(t == 0), stop=(t == NKT - 1))
                nc.tensor.matmul(pAB[1][:], v_s[:, t, 2 * p + 1, 0:DH + 1],
                                 pt[:, 512:1024],
                                 start=(t == 0), stop=(t == NKT - 1))
                if t == NKT - 1:
                    normalize(w)
                    if p == 1:
                        c_ops(qb)

            # tail filler: keep the PE HAM-warm through the final normalize so
            # the last C matmuls run at full clock
            with tc.high_priority(offset=LOW2):
                for i in range(6):
                    tg = "pA" if i % 2 == 0 else "pB"
                    wps = pav.tile([128, 512], F32, tag=tg, name=f"twarm{i}")
                    nc.tensor.matmul(wps[:], warm[:, 0:128], warm[:],
                                     start=True, stop=True)
    nc.compile()
    return nc


def shard_inputs(x, W_qkv, b_qkv, W_out, b_out=None):
    """Build the 8 per-core input maps. Core c: batch c//4, head group c%4."""
    in_maps = []
    scale = 1.0 / np.sqrt(np.float32(DH))
    bf16 = ml_dtypes.bfloat16
    for c in range(8):
        b, g = divmod(c, 4)
        cs = slice(g * 256, g * 256 + 256)
        xTc = np.ascontiguousarray(x[b].T)                       # [D, S]
        wq = W_qkv[:, 0:D][:, cs] * scale                        # [D, 256]
        wk = W_qkv[:, D:2 * D][:, cs]
        wqkc = np.ascontiguousarray(np.concatenate([wq, wk], axis=1))  # [D, 512]
        bq = b_qkv[0:D][cs] * scale
        bk = b_qkv[D:2 * D][cs]
        bqkc = np.concatenate([bq, bk]).reshape(CQK // 128, 128).T     # [128, 4]
        wvc = np.ascontiguousarray(W_qkv[:, 2 * D:3 * D][:, cs])       # [D, 256]
        bvbc = np.ascontiguousarray(
            np.broadcast_to(b_qkv[2 * D:3 * D][cs], (128, CV)))        # [128, 256]
        woutc = np.ascontiguousarray(W_out[cs, :])                     # [256, D]
        in_maps.append({
            "xT": xTc.astype(bf16),
            "wqk": wqkc.astype(bf16),
            "bqk": np.ascontiguousarray(bqkc).astype(np.float32),
            "wv": wvc.astype(bf16),
            "bvb": bvbc.astype(bf16),
            "wout": woutc.astype(bf16),
        })
    return in_maps


_NC_CACHE = []


def _get_nc():
    if not _NC_CACHE:
        _NC_CACHE.append(build_kernel())
    return _NC_CACHE[0]


def run_sharded(in_maps, **kwargs):
    nc = _get_nc()
    return run_bass_kernel_spmd(nc, in_maps, core_ids=list(range(8)), **kwargs)


def gather_output(results, b_out):
    out = np.empty((B, S, D), dtype=np.float32)
    for b in range(B):
        acc = np.asarray(results[4 * b]["outT"], dtype=np.float32).copy()
        for g in range(1, 4):
            acc += np.asarray(results[4 * b + g]["outT"], dtype=np.float32)
        out[b] = acc.T + b_out[None, :]
    return out


def kernel(x, W_qkv, b_qkv, W_out, b_out):
    x = np.asarray(x, dtype=np.float32)
    W_qkv = np.asarray(W_qkv, dtype=np.float32)
    b_qkv = np.asarray(b_qkv, dtype=np.float32)
    W_out = np.asarray(W_out, dtype=np.float32)
    b_out = np.asarray(b_out, dtype=np.float32)
    in_maps = shard_inputs(x=x, W_qkv=W_qkv, b_qkv=b_qkv, W_out=W_out, b_out=b_out)
    res = run_sharded(in_maps)
    return gather_output(res.results, b_out)
